# revision 2
# baseline (speedup 1.0000x reference)
"""Modulated deformable conv v2 (B=8, C=O=256, H=W=64, 3x3) on 8 trn2 NeuronCores.

v2.1: bf16 datapath + host-precomputed gather indices / corner weights.

Strategy: data-parallel over batch (1 image per core). Per core:
  - host marshals the image into a "patch array" in3[j] = concat of padded
    [HW, C] rows (j, j+1, j+64, j+65) in bf16, so one contiguous 2KB DMA
    descriptor fetches the full 2x2 bilinear patch for all 256 channels.
  - host also computes the int16 gather row indices (jw, 16-partition
    wrapped, iteration-major) and the 4 folded corner weights (bilinear
    frac * validity * modulation mask, W-layout bf16).
  - device: gpsimd dma_gather streams patch rows (HBM->SBUF) 4 iterations
    (2048 rows) per call, DVE applies the 4 per-row corner weights
    (1 tensor_scalar + 3 scalar_tensor_tensor per 128-position group, all
    bf16, no drains), PE transposes [p,c]->[c,p] via identity matmuls
    (bf16 PSUM), ACT copies PSUM->SBUF into per-chunk [128, 9*1024] tiles,
    PE runs one N=512 einsum per 512-position chunk with PSUM
    accumulation over (k, cb), ACT folds the bias, HWDGE DMAs f32 output.
"""

import numpy as np
from contextlib import ExitStack

import concourse.bacc as bacc
import concourse.bass as bass
import concourse.mybir as mybir
from concourse import bass_utils
from concourse.library_config import mlp

AP = bass.AP
F32 = mybir.dt.float32
BF16 = mybir.dt.bfloat16
I16 = mybir.dt.int16

# problem constants (hardcoded per contract)
B = 8
C = 256
O = 256
H = W = 64
HW = 4096
K2 = 9

# tiling
NCH = 8          # spatial chunks
CHP = 512        # positions per chunk
NG = 4           # 128-position groups per chunk
NITER = NCH * K2 # 72 macro iterations (chunk-major, then k)
NGI = NITER * NG # 288 (i,g) steps
GB = 2           # iterations per dma_gather batch
NBATCH = NITER // GB  # 18

PADLO = 65       # leading pad rows in the padded [HW, C] image
R2 = 4292        # padded image rows
R3 = 4232        # patch-array rows (4225 used)


def _np_bf16():
    import ml_dtypes
    return ml_dtypes.bfloat16


# ---------------------------------------------------------------------------
# bass program
# ---------------------------------------------------------------------------

def build_nc():
    nc = bacc.Bacc("TRN2", detect_race_conditions=False)

    in3 = nc.dram_tensor("in3", [R3, 1024], BF16, kind="ExternalInput")
    wT = nc.dram_tensor("wT", [2304, 256], BF16, kind="ExternalInput")
    identm = nc.dram_tensor("identm", [128, 128], BF16, kind="ExternalInput")
    biasm = nc.dram_tensor("biasm", [128, 2], F32, kind="ExternalInput")
    jwH = nc.dram_tensor("jwH", [128, 2304], I16, kind="ExternalInput")
    wcH = nc.dram_tensor("wcH", [128, 4 * 288], F32, kind="ExternalInput")
    outT = nc.dram_tensor("out", [256, 4096], F32, kind="ExternalOutput")

    with ExitStack() as ctx:
        ec = ctx.enter_context

        # sbuf
        gdst = [ec(nc.sbuf_tensor(f"gdst{j}", [128, GB * 4096], BF16))
                for j in range(2)]
        s_rows = [ec(nc.sbuf_tensor(f"srows{j}", [128, 1024], BF16)) for j in range(2)]
        st_sb = [ec(nc.sbuf_tensor(f"stsb{j}", [128, 256], BF16)) for j in range(2)]
        out_sb = [ec(nc.sbuf_tensor(f"outsb{j}", [128, 1024], F32)) for j in range(2)]
        w_sb = ec(nc.sbuf_tensor("wsb", [128, 4608], BF16))
        ident_sb = ec(nc.sbuf_tensor("identsb", [128, 128], BF16))
        bias_sb = ec(nc.sbuf_tensor("biassb", [128, 2], F32))
        tmpA = [ec(nc.sbuf_tensor(f"tmpA{g}", [128, 256], BF16)) for g in range(NG)]
        tmpB = [ec(nc.sbuf_tensor(f"tmpB{g}", [128, 256], BF16)) for g in range(NG)]
        wcor = [ec(nc.sbuf_tensor(f"wc{q}", [128, 288], F32)) for q in range(4)]
        jw = ec(nc.sbuf_tensor("jw", [128, 2304], I16))

        # psum: transposes in bf16 (must match lhsT dtype), einsum accum f32
        psT = [ec(nc.psum_tensor(f"psT{j}", [128, 1024], BF16)) for j in range(2)]
        psE = [ec(nc.psum_tensor(f"psE{g}", [128, 512], F32)) for g in range(NG)]

        sem_ld = ec(nc.semaphore("sem_ld"))
        sem_prep2 = ec(nc.semaphore("sem_prep2"))
        sem_gat = ec(nc.semaphore("sem_gat"))
        sem_dve = ec(nc.semaphore("sem_dve"))
        sem_pet = ec(nc.semaphore("sem_pet"))
        sem_act = ec(nc.semaphore("sem_act"))
        sem_pee = ec(nc.semaphore("sem_pee"))
        sem_epi = ec(nc.semaphore("sem_epi"))
        sem_out = ec(nc.semaphore("sem_out"))

        # ---- AP helpers (flat element offsets) ----
        def sb(t, off, free, count=128, pstep=None):
            if pstep is None:
                pstep = t.shape[1] if len(t.shape) == 2 else int(np.prod(t.shape[1:]))
            return AP(t, off, [[pstep, count], [1, free]])

        def scl(t, col):
            return AP(t, col, [[t.shape[1], 128], [1, 1]])

        loads = [
            (sb(ident_sb, 0, 128), AP(identm, 0, [[128, 128], [1, 128]])),
            (sb(bias_sb, 0, 2), AP(biasm, 0, [[2, 128], [1, 2]])),
            (sb(jw, 0, 2304), AP(jwH, 0, [[2304, 128], [1, 2304]])),
        ]
        for q in range(4):
            loads.append((sb(wcor[q], 0, 288),
                          AP(wcH, q * 288, [[4 * 288, 128], [1, 288]])))
        for kcb in range(18):
            loads.append((sb(w_sb, kcb * 256, 256),
                          AP(wT, kcb * 128 * 256, [[256, 128], [1, 256]])))
        n_loads = len(loads)

        glast = {}  # chunk -> gi of last transpose-copy step
        for c in range(NCH):
            glast[(c * K2 + (K2 - 1)) * NG + (NG - 1)] = c

        with nc.Block() as block:

            @block.sync
            def _(sync):
                for dst, src in loads:
                    sync.dma_start(dst, src).then_inc(sem_ld, 16)
                for c in range(NCH):
                    sync.wait_ge(sem_epi, 8 * (c + 1))
                    for ob in range(2):
                        dst = AP(outT, ob * 128 * 4096 + c * 512,
                                 [[4096, 128], [1, 512]])
                        src = sb(out_sb[c % 2], ob * 512, 512)
                        sync.dma_start(dst, src).then_inc(sem_out, 16)

            @block.gpsimd
            def _(gp):
                gp.load_library(mlp)
                gp.wait_ge(sem_ld, 16 * n_loads)
                in3_ap = AP(in3, 0, [[1024, R3], [1, 1024]])
                for b in range(NBATCH):
                    if b >= 2:
                        # gdst[b%2] reuse: DVE done with iters of batch b-2
                        gp.wait_ge(sem_dve, 4 * GB * (b - 1))
                    dst = AP(gdst[b % 2], 0,
                             [[GB * 4096, 128], [1024, 4 * GB], [1, 1024]])
                    idx = AP(jw, b * GB * 32, [[2304, 128], [1, GB * 32]])
                    gp.dma_gather(dst, in3_ap, idx, GB * CHP, GB * CHP, 1024,
                                  prepare_only=True, sem=sem_gat).then_inc(
                        sem_prep2, 1)
                    gp.wait_ge(sem_prep2, b + 1)
                    gp.trigger_dma(count=1)

            @block.vector
            def _(v):
                A = mybir.AluOpType
                v.wait_ge(sem_ld, 16 * n_loads)

                # ---- per-(i,g) corner-weight application ----
                # 4 independent groups per stage; RAW pairs are 3 ops apart,
                # so no explicit drains needed (DVE pipe is 8 slices deep).
                for i in range(NITER):
                    c, k = divmod(i, K2)
                    v.wait_ge(sem_gat, 16 * (i // GB + 1))
                    if i >= 2:
                        v.wait_ge(sem_pet, NG * (i - 1))
                    cols = [k * 32 + c * NG + g for g in range(NG)]
                    base = (i % GB) * 4096
                    gsl = [[AP(gdst[(i // GB) % 2], base + g * 1024 + q * 256,
                               [[GB * 4096, 128], [1, 256]]) for q in range(4)]
                           for g in range(NG)]
                    a_ = [sb(t, 0, 256) for t in tmpA]
                    b_ = [sb(t, 0, 256) for t in tmpB]
                    dstS = [sb(s_rows[i % 2], g * 256, 256) for g in range(NG)]
                    for g in range(NG):
                        v.tensor_single_scalar(a_[g], gsl[g][0],
                                               scl(wcor[0], cols[g]), A.mult)
                    for g in range(NG):
                        v.scalar_tensor_tensor(b_[g], gsl[g][1],
                                               scl(wcor[1], cols[g]), a_[g],
                                               A.mult, A.add)
                    for g in range(NG):
                        v.scalar_tensor_tensor(a_[g], gsl[g][2],
                                               scl(wcor[2], cols[g]), b_[g],
                                               A.mult, A.add)
                    for g in range(NG):
                        v.scalar_tensor_tensor(dstS[g], gsl[g][3],
                                               scl(wcor[3], cols[g]), a_[g],
                                               A.mult, A.add).then_inc(sem_dve)

            @block.tensor
            def _(te):
                te.wait_ge(sem_ld, 16 * n_loads)

                def emit_einsum(gg):
                    i2, g2 = divmod(gg, NG)
                    c2, k2 = divmod(i2, K2)
                    te.wait_ge(sem_act, gg + 1)
                    if k2 == 0 and c2 >= 1:
                        # psE[g2] bank reused across chunks; wait for the
                        # previous chunk's epilogue to finish reading it
                        te.wait_ge(sem_epi, 8 * c2)
                    last = None
                    for ob in range(2):
                        for cb in range(2):
                            lhs = AP(w_sb, (k2 * 2 + cb) * 256 + ob * 128,
                                     [[4608, 128], [1, 128]])
                            rhs = AP(st_sb[gg % 2], cb * 128, [[256, 128], [1, 128]])
                            dst = AP(psE[g2], ob * 128, [[512, 128], [1, 128]])
                            last = te.matmul(dst, lhs, rhs,
                                             start=(k2 == 0 and ob == 0
                                                    and cb == 0),
                                             stop=(k2 == K2 - 1 and ob == 1
                                                   and cb == 1))
                    last.then_inc(sem_pee)

                for i in range(NITER):
                    for g in range(NG):
                        gi = NG * i + g
                        te.wait_ge(sem_dve, gi + 1)
                        if gi >= 2:
                            te.wait_ge(sem_act, gi - 1)
                        pg = gi % 2
                        last = None
                        for cb in range(2):
                            src = AP(s_rows[i % 2], g * 256 + cb * 128,
                                     [[1024, 128], [1, 128]])
                            dst = AP(psT[pg], cb * 128, [[1024, 128], [1, 128]])
                            last = te.transpose(dst, src, sb(ident_sb, 0, 128))
                        last.then_inc(sem_pet)
                        if gi >= 1:
                            emit_einsum(gi - 1)
                emit_einsum(NGI - 1)

            @block.scalar
            def _(sc):
                IDENT = mybir.ActivationFunctionType.Identity
                for gi in range(NGI):
                    sc.wait_ge(sem_pet, gi + 1)
                    if gi >= 2:
                        sc.wait_ge(sem_pee, gi - 1)
                    sc.activation(sb(st_sb[gi % 2], 0, 256),
                                  AP(psT[gi % 2], 0, [[1024, 128], [1, 256]]),
                                  IDENT).then_inc(sem_act)
                    c = glast.get(gi)
                    if c is not None:
                        if c >= 2:
                            sc.wait_ge(sem_out, 32 * (c - 1))
                        for g2 in range(NG):
                            sc.wait_ge(sem_pee, (c * K2 + K2 - 1) * NG + g2 + 1)
                            for ob in range(2):
                                sc.activation(
                                    sb(out_sb[c % 2], ob * 512 + g2 * 128, 128),
                                    AP(psE[g2], ob * 128,
                                       [[512, 128], [1, 128]]),
                                    IDENT, bias=scl(bias_sb, ob),
                                ).then_inc(sem_epi)

    nc.compile()
    return nc


# ---------------------------------------------------------------------------
# host marshalling
# ---------------------------------------------------------------------------

def _to_W(f):
    # f [9, 4096] -> [128, 288]; fW[p%128, k*32 + p//128] = f[k, p]
    return np.ascontiguousarray(
        f.reshape(9, 32, 128).transpose(2, 0, 1).reshape(128, 288))


def _to_I72(f):
    # f [72, 512] (iter-major, i = c*9+k) -> wrapped [128, 2304];
    # fI[r, i*32+t] = f[i, t*16 + r%16]
    a = f.reshape(72, 32, 16).transpose(2, 0, 1).reshape(16, 2304)
    return np.ascontiguousarray(np.tile(a, (8, 1)))


def marshal(inputs):
    bf16 = _np_bf16()
    inp = np.asarray(inputs["input"], np.float32)
    off = np.asarray(inputs["offset"], np.float32)
    msk = np.asarray(inputs["mask"], np.float32)
    wgt = np.asarray(inputs["weight"], np.float32)
    bias = np.asarray(inputs["bias"], np.float32)

    wT = np.ascontiguousarray(
        wgt.reshape(O, C, K2).transpose(2, 1, 0).reshape(2304, 256)).astype(bf16)
    identm = np.eye(128, dtype=np.float32).astype(bf16)
    biasm = np.ascontiguousarray(bias.reshape(2, 128).T)

    # sample coordinates: y = off_y + base_y, base_y = ho - 1 + k//3
    ho = np.arange(HW, dtype=np.float32) // 64
    wo = np.arange(HW, dtype=np.float32) % 64
    ks = np.arange(K2, dtype=np.float32)
    by = ho[None, :] - 1.0 + (ks // 3)[:, None]     # [9, 4096]
    bx = wo[None, :] - 1.0 + (ks % 3)[:, None]

    in_maps = []
    for b in range(B):
        img = inp[b].transpose(1, 2, 0).reshape(HW, C)
        in2p = np.zeros((R2, C), np.float32)
        in2p[PADLO:PADLO + HW] = img
        in3 = np.zeros((R3, 1024), np.float32)
        n = HW + 2 * PADLO - 1  # 4225 usable rows
        in3[:n, 0:256] = in2p[0:n]
        in3[:n, 256:512] = in2p[1:n + 1]
        in3[:n, 512:768] = in2p[64:n + 64]
        in3[:n, 768:1024] = in2p[65:n + 65]

        off_y = off[b, 0::2].reshape(K2, HW)
        off_x = off[b, 1::2].reshape(K2, HW)
        m = msk[b].reshape(K2, HW)

        y = off_y + by
        x = off_x + bx
        y0 = np.floor(y)
        x0 = np.floor(x)
        ly = (y - y0).astype(np.float32)
        lx = (x - x0).astype(np.float32)
        hy = 1.0 - ly
        hx = 1.0 - lx
        vy0 = ((y0 >= 0) & (y0 <= 63)).astype(np.float32)
        vy1 = ((y0 >= -1) & (y0 <= 62)).astype(np.float32)
        vx0 = ((x0 >= 0) & (x0 <= 63)).astype(np.float32)
        vx1 = ((x0 >= -1) & (x0 <= 62)).astype(np.float32)
        w00 = hy * hx * vy0 * vx0 * m
        w01 = hy * lx * vy0 * vx1 * m
        w10 = ly * hx * vy1 * vx0 * m
        w11 = ly * lx * vy1 * vx1 * m

        py = np.clip(y0, -1, 63)
        px = np.clip(x0, -1, 63)
        J = (64.0 * py + px + float(PADLO)).astype(np.float32)
        # iteration-major [72, 512] with i = c*9 + k
        J72 = J.reshape(9, 8, 512).transpose(1, 0, 2).reshape(72, 512)

        wcH = np.concatenate(
            [_to_W(w) for w in (w00, w01, w10, w11)], axis=1)
        im = {
            "in3": in3.astype(bf16),
            "jwH": _to_I72(J72).astype(np.int16),
            "wcH": np.ascontiguousarray(wcH, np.float32),
            "wT": wT, "identm": identm, "biasm": biasm,
        }
        in_maps.append(im)
    return in_maps


_NC_CACHE = {}


def _get_nc():
    if "nc" not in _NC_CACHE:
        _NC_CACHE["nc"] = build_nc()
    return _NC_CACHE["nc"]


def run(inputs, trace=False, **kw):
    nc = _get_nc()
    in_maps = marshal(inputs)
    res = bass_utils.run_bass_kernel_spmd(nc, in_maps, core_ids=list(range(B)),
                                          trace=trace, **kw)
    out = np.stack([r["out"].reshape(O, H, W) for r in res.results])
    return out.astype(np.float32), res


def kernel(**inputs):
    return run(inputs)[0]


# revision 3
# speedup vs baseline: 1.1921x; 1.1921x over previous
"""Modulated deformable conv v2 (B=8, C=O=256, H=W=64, 3x3) on 8 trn2 NeuronCores.

v2.1: bf16 datapath + host-precomputed gather indices / corner weights.

Strategy: data-parallel over batch (1 image per core). Per core:
  - host marshals the image into a "patch array" in3[j] = concat of padded
    [HW, C] rows (j, j+1, j+64, j+65) in bf16, so one contiguous 2KB DMA
    descriptor fetches the full 2x2 bilinear patch for all 256 channels.
  - host also computes the int16 gather row indices (jw, 16-partition
    wrapped, iteration-major) and the 4 folded corner weights (bilinear
    frac * validity * modulation mask, W-layout bf16).
  - device: gpsimd dma_gather streams patch rows (HBM->SBUF) 4 iterations
    (2048 rows) per call, DVE applies the 4 per-row corner weights
    (1 tensor_scalar + 3 scalar_tensor_tensor per 128-position group, all
    bf16, no drains), PE transposes [p,c]->[c,p] via identity matmuls
    (bf16 PSUM), ACT copies PSUM->SBUF into per-chunk [128, 9*1024] tiles,
    PE runs one N=512 einsum per 512-position chunk with PSUM
    accumulation over (k, cb), ACT folds the bias, HWDGE DMAs f32 output.
"""

import numpy as np
from contextlib import ExitStack

import concourse.bacc as bacc
import concourse.bass as bass
import concourse.mybir as mybir
from concourse import bass_utils
from concourse.library_config import mlp

AP = bass.AP
F32 = mybir.dt.float32
BF16 = mybir.dt.bfloat16
I16 = mybir.dt.int16

# problem constants (hardcoded per contract)
B = 8
C = 256
O = 256
H = W = 64
HW = 4096
K2 = 9

# tiling
NCH = 8          # spatial chunks
CHP = 512        # positions per chunk
NG = 4           # 128-position groups per chunk
NITER = NCH * K2 # 72 macro iterations (chunk-major, then k)
NGI = NITER * NG # 288 (i,g) steps
GB = 2           # iterations per dma_gather batch
NBATCH = NITER // GB  # 18

PADLO = 65       # leading pad rows in the padded [HW, C] image
R2 = 4292        # padded image rows
R3 = 4232        # patch-array rows (4225 used)


def _np_bf16():
    import ml_dtypes
    return ml_dtypes.bfloat16


# ---------------------------------------------------------------------------
# bass program
# ---------------------------------------------------------------------------

def build_nc():
    nc = bacc.Bacc("TRN2", detect_race_conditions=False)

    in3 = nc.dram_tensor("in3", [R3, 1024], BF16, kind="ExternalInput")
    wT = nc.dram_tensor("wT", [2304, 256], BF16, kind="ExternalInput")
    identm = nc.dram_tensor("identm", [128, 128], BF16, kind="ExternalInput")
    biasm = nc.dram_tensor("biasm", [128, 2], F32, kind="ExternalInput")
    jwH = nc.dram_tensor("jwH", [128, 2304], I16, kind="ExternalInput")
    wcH = nc.dram_tensor("wcH", [128, 4 * 288], F32, kind="ExternalInput")
    outT = nc.dram_tensor("out", [256, 4096], F32, kind="ExternalOutput")

    with ExitStack() as ctx:
        ec = ctx.enter_context

        # sbuf
        gdst = [ec(nc.sbuf_tensor(f"gdst{j}", [128, GB * 4096], BF16))
                for j in range(2)]
        s_rows = [ec(nc.sbuf_tensor(f"srows{j}", [128, 1024], BF16)) for j in range(2)]
        st_sb = [ec(nc.sbuf_tensor(f"stsb{j}", [128, 256], BF16)) for j in range(2)]
        out_sb = [ec(nc.sbuf_tensor(f"outsb{j}", [128, 1024], F32)) for j in range(2)]
        w_sb = ec(nc.sbuf_tensor("wsb", [128, 4608], BF16))
        ident_sb = ec(nc.sbuf_tensor("identsb", [128, 128], BF16))
        bias_sb = ec(nc.sbuf_tensor("biassb", [128, 2], F32))
        tmpA = [[ec(nc.sbuf_tensor(f"tmpA{j}_{g}", [128, 256], BF16))
                 for g in range(NG)] for j in range(3)]
        tmpB = [ec(nc.sbuf_tensor(f"tmpB{g}", [128, 256], BF16)) for g in range(NG)]
        wcor = [ec(nc.sbuf_tensor(f"wc{q}", [128, 288], F32)) for q in range(4)]
        jw = ec(nc.sbuf_tensor("jw", [128, 2304], I16))

        # psum: transposes in bf16 (must match lhsT dtype), einsum accum f32
        psT = [ec(nc.psum_tensor(f"psT{j}", [128, 1024], BF16)) for j in range(2)]
        psE = [ec(nc.psum_tensor(f"psE{g}", [128, 512], F32)) for g in range(NG)]

        sem_ld = ec(nc.semaphore("sem_ld"))
        sem_ac0 = ec(nc.semaphore("sem_ac0"))
        sem_prep2 = ec(nc.semaphore("sem_prep2"))
        sem_gat = ec(nc.semaphore("sem_gat"))
        sem_dve = ec(nc.semaphore("sem_dve"))
        sem_pet = ec(nc.semaphore("sem_pet"))
        sem_act = ec(nc.semaphore("sem_act"))
        sem_pee = ec(nc.semaphore("sem_pee"))
        sem_epi = ec(nc.semaphore("sem_epi"))
        sem_out = ec(nc.semaphore("sem_out"))

        # ---- AP helpers (flat element offsets) ----
        def sb(t, off, free, count=128, pstep=None):
            if pstep is None:
                pstep = t.shape[1] if len(t.shape) == 2 else int(np.prod(t.shape[1:]))
            return AP(t, off, [[pstep, count], [1, free]])

        def scl(t, col):
            return AP(t, col, [[t.shape[1], 128], [1, 1]])

        loads = [
            (sb(ident_sb, 0, 128), AP(identm, 0, [[128, 128], [1, 128]])),
            (sb(bias_sb, 0, 2), AP(biasm, 0, [[2, 128], [1, 2]])),
            (sb(jw, 0, 2304), AP(jwH, 0, [[2304, 128], [1, 2304]])),
        ]
        for q in range(4):
            loads.append((sb(wcor[q], 0, 288),
                          AP(wcH, q * 288, [[4 * 288, 128], [1, 288]])))
        for kcb in range(18):
            loads.append((sb(w_sb, kcb * 256, 256),
                          AP(wT, kcb * 128 * 256, [[256, 128], [1, 256]])))
        n_loads = len(loads)

        glast = {}  # chunk -> gi of last transpose-copy step
        for c in range(NCH):
            glast[(c * K2 + (K2 - 1)) * NG + (NG - 1)] = c

        with nc.Block() as block:

            @block.sync
            def _(sync):
                for dst, src in loads:
                    sync.dma_start(dst, src).then_inc(sem_ld, 16)
                for c in range(NCH):
                    sync.wait_ge(sem_epi, 8 * (c + 1))
                    for ob in range(2):
                        dst = AP(outT, ob * 128 * 4096 + c * 512,
                                 [[4096, 128], [1, 512]])
                        src = sb(out_sb[c % 2], ob * 512, 512)
                        sync.dma_start(dst, src).then_inc(sem_out, 16)

            @block.gpsimd
            def _(gp):
                gp.load_library(mlp)
                gp.wait_ge(sem_ld, 16 * n_loads)
                in3_ap = AP(in3, 0, [[1024, R3], [1, 1024]])
                for b in range(NBATCH):
                    if b >= 2:
                        # gdst[b%2] reuse: DVE done with iters of batch b-2
                        gp.wait_ge(sem_dve, 4 * GB * (b - 1))
                    dst = AP(gdst[b % 2], 0,
                             [[GB * 4096, 128], [1024, 4 * GB], [1, 1024]])
                    idx = AP(jw, b * GB * 32, [[2304, 128], [1, GB * 32]])
                    gp.dma_gather(dst, in3_ap, idx, GB * CHP, GB * CHP, 1024,
                                  prepare_only=True, sem=sem_gat).then_inc(
                        sem_prep2, 1)
                    gp.wait_ge(sem_prep2, b + 1)
                    gp.trigger_dma(count=1)

            @block.vector
            def _(v):
                A = mybir.AluOpType
                v.wait_ge(sem_ld, 16 * n_loads)

                # ---- per-(i,g) corner-weight application ----
                # 4 independent groups per stage; RAW pairs are 3 ops apart,
                # so no explicit drains needed (DVE pipe is 8 slices deep).
                for i in range(NITER):
                    c, k = divmod(i, K2)
                    v.wait_ge(sem_gat, 16 * (i // GB + 1))
                    if i >= 2:
                        v.wait_ge(sem_pet, NG * (i - 1))
                    cols = [k * 32 + c * NG + g for g in range(NG)]
                    base = (i % GB) * 4096
                    gsl = [[AP(gdst[(i // GB) % 2], base + g * 1024 + q * 256,
                               [[GB * 4096, 128], [1, 256]]) for q in range(4)]
                           for g in range(NG)]
                    a_ = [sb(t, 0, 256) for t in tmpA[i % 3]]
                    b_ = [sb(t, 0, 256) for t in tmpB]
                    dstS = [sb(s_rows[i % 2], g * 256, 256) for g in range(NG)]
                    v.wait_ge(sem_ac0, i + 1)
                    for g in range(NG):
                        v.scalar_tensor_tensor(b_[g], gsl[g][1],
                                               scl(wcor[1], cols[g]), a_[g],
                                               A.mult, A.add)
                    for g in range(NG):
                        v.scalar_tensor_tensor(a_[g], gsl[g][2],
                                               scl(wcor[2], cols[g]), b_[g],
                                               A.mult, A.add)
                    for g in range(NG):
                        v.scalar_tensor_tensor(dstS[g], gsl[g][3],
                                               scl(wcor[3], cols[g]), a_[g],
                                               A.mult, A.add).then_inc(sem_dve)

            @block.tensor
            def _(te):
                te.wait_ge(sem_ld, 16 * n_loads)

                def emit_einsum(gg):
                    i2, g2 = divmod(gg, NG)
                    c2, k2 = divmod(i2, K2)
                    te.wait_ge(sem_act, gg + 1)
                    if k2 == 0 and c2 >= 1:
                        # psE[g2] bank reused across chunks; wait for the
                        # previous chunk's epilogue to finish reading it
                        te.wait_ge(sem_epi, 8 * c2)
                    last = None
                    for ob in range(2):
                        for cb in range(2):
                            lhs = AP(w_sb, (k2 * 2 + cb) * 256 + ob * 128,
                                     [[4608, 128], [1, 128]])
                            rhs = AP(st_sb[gg % 2], cb * 128, [[256, 128], [1, 128]])
                            dst = AP(psE[g2], ob * 128, [[512, 128], [1, 128]])
                            last = te.matmul(dst, lhs, rhs,
                                             start=(k2 == 0 and ob == 0
                                                    and cb == 0),
                                             stop=(k2 == K2 - 1 and ob == 1
                                                   and cb == 1))
                    last.then_inc(sem_pee)

                for i in range(NITER):
                    for g in range(NG):
                        gi = NG * i + g
                        te.wait_ge(sem_dve, gi + 1)
                        if gi >= 2:
                            te.wait_ge(sem_act, gi - 1)
                        pg = gi % 2
                        last = None
                        for cb in range(2):
                            src = AP(s_rows[i % 2], g * 256 + cb * 128,
                                     [[1024, 128], [1, 128]])
                            dst = AP(psT[pg], cb * 128, [[1024, 128], [1, 128]])
                            last = te.transpose(dst, src, sb(ident_sb, 0, 128))
                        last.then_inc(sem_pet)
                        if gi >= 1:
                            emit_einsum(gi - 1)
                emit_einsum(NGI - 1)

            @block.scalar
            def _(sc):
                IDENT = mybir.ActivationFunctionType.Identity

                def corner0(j):
                    # a_g = gathered corner-0 block * w00[p], into tmpA[j%2]
                    cj, kj = divmod(j, K2)
                    sc.wait_ge(sem_gat, 16 * (j // GB + 1))
                    if j >= 3:
                        # tmpA[j%3] reuse: DVE stage1 of iter j-3 done
                        sc.wait_ge(sem_dve, NG * (j - 2))
                    a0 = None
                    for g0 in range(NG):
                        col0 = kj * 32 + cj * NG + g0
                        src0 = AP(gdst[(j // GB) % 2],
                                  (j % GB) * 4096 + g0 * 1024,
                                  [[GB * 4096, 128], [1, 256]])
                        a0 = sc.activation(sb(tmpA[j % 3][g0], 0, 256),
                                           src0, IDENT,
                                           scale=scl(wcor[0], col0))
                    a0.then_inc(sem_ac0)

                corner0(0)
                corner0(1)
                for gi in range(NGI):
                    i, g = divmod(gi, NG)
                    sc.wait_ge(sem_pet, gi + 1)
                    if gi >= 2:
                        sc.wait_ge(sem_pee, gi - 1)
                    sc.activation(sb(st_sb[gi % 2], 0, 256),
                                  AP(psT[gi % 2], 0, [[1024, 128], [1, 256]]),
                                  IDENT).then_inc(sem_act)
                    if g == NG - 1 and i + 2 < NITER:
                        # corner-0 two iters ahead, BEFORE any epilogue wait
                        corner0(i + 2)
                    c = glast.get(gi)
                    if c is not None:
                        if c >= 2:
                            sc.wait_ge(sem_out, 32 * (c - 1))
                        for g2 in range(NG):
                            sc.wait_ge(sem_pee, (c * K2 + K2 - 1) * NG + g2 + 1)
                            for ob in range(2):
                                sc.activation(
                                    sb(out_sb[c % 2], ob * 512 + g2 * 128, 128),
                                    AP(psE[g2], ob * 128,
                                       [[512, 128], [1, 128]]),
                                    IDENT, bias=scl(bias_sb, ob),
                                ).then_inc(sem_epi)

    nc.compile()
    return nc


# ---------------------------------------------------------------------------
# host marshalling
# ---------------------------------------------------------------------------

def _to_W(f):
    # f [9, 4096] -> [128, 288]; fW[p%128, k*32 + p//128] = f[k, p]
    return np.ascontiguousarray(
        f.reshape(9, 32, 128).transpose(2, 0, 1).reshape(128, 288))


def _to_I72(f):
    # f [72, 512] (iter-major, i = c*9+k) -> wrapped [128, 2304];
    # fI[r, i*32+t] = f[i, t*16 + r%16]
    a = f.reshape(72, 32, 16).transpose(2, 0, 1).reshape(16, 2304)
    return np.ascontiguousarray(np.tile(a, (8, 1)))


def marshal(inputs):
    bf16 = _np_bf16()
    inp = np.asarray(inputs["input"], np.float32)
    off = np.asarray(inputs["offset"], np.float32)
    msk = np.asarray(inputs["mask"], np.float32)
    wgt = np.asarray(inputs["weight"], np.float32)
    bias = np.asarray(inputs["bias"], np.float32)

    wT = np.ascontiguousarray(
        wgt.reshape(O, C, K2).transpose(2, 1, 0).reshape(2304, 256)).astype(bf16)
    identm = np.eye(128, dtype=np.float32).astype(bf16)
    biasm = np.ascontiguousarray(bias.reshape(2, 128).T)

    # sample coordinates: y = off_y + base_y, base_y = ho - 1 + k//3
    ho = np.arange(HW, dtype=np.float32) // 64
    wo = np.arange(HW, dtype=np.float32) % 64
    ks = np.arange(K2, dtype=np.float32)
    by = ho[None, :] - 1.0 + (ks // 3)[:, None]     # [9, 4096]
    bx = wo[None, :] - 1.0 + (ks % 3)[:, None]

    in_maps = []
    for b in range(B):
        img = inp[b].transpose(1, 2, 0).reshape(HW, C)
        in2p = np.zeros((R2, C), np.float32)
        in2p[PADLO:PADLO + HW] = img
        in3 = np.zeros((R3, 1024), np.float32)
        n = HW + 2 * PADLO - 1  # 4225 usable rows
        in3[:n, 0:256] = in2p[0:n]
        in3[:n, 256:512] = in2p[1:n + 1]
        in3[:n, 512:768] = in2p[64:n + 64]
        in3[:n, 768:1024] = in2p[65:n + 65]

        off_y = off[b, 0::2].reshape(K2, HW)
        off_x = off[b, 1::2].reshape(K2, HW)
        m = msk[b].reshape(K2, HW)

        y = off_y + by
        x = off_x + bx
        y0 = np.floor(y)
        x0 = np.floor(x)
        ly = (y - y0).astype(np.float32)
        lx = (x - x0).astype(np.float32)
        hy = 1.0 - ly
        hx = 1.0 - lx
        vy0 = ((y0 >= 0) & (y0 <= 63)).astype(np.float32)
        vy1 = ((y0 >= -1) & (y0 <= 62)).astype(np.float32)
        vx0 = ((x0 >= 0) & (x0 <= 63)).astype(np.float32)
        vx1 = ((x0 >= -1) & (x0 <= 62)).astype(np.float32)
        w00 = hy * hx * vy0 * vx0 * m
        w01 = hy * lx * vy0 * vx1 * m
        w10 = ly * hx * vy1 * vx0 * m
        w11 = ly * lx * vy1 * vx1 * m

        py = np.clip(y0, -1, 63)
        px = np.clip(x0, -1, 63)
        J = (64.0 * py + px + float(PADLO)).astype(np.float32)
        # iteration-major [72, 512] with i = c*9 + k
        J72 = J.reshape(9, 8, 512).transpose(1, 0, 2).reshape(72, 512)

        wcH = np.concatenate(
            [_to_W(w) for w in (w00, w01, w10, w11)], axis=1)
        im = {
            "in3": in3.astype(bf16),
            "jwH": _to_I72(J72).astype(np.int16),
            "wcH": np.ascontiguousarray(wcH, np.float32),
            "wT": wT, "identm": identm, "biasm": biasm,
        }
        in_maps.append(im)
    return in_maps


_NC_CACHE = {}


def _get_nc():
    if "nc" not in _NC_CACHE:
        _NC_CACHE["nc"] = build_nc()
    return _NC_CACHE["nc"]


def run(inputs, trace=False, **kw):
    nc = _get_nc()
    in_maps = marshal(inputs)
    res = bass_utils.run_bass_kernel_spmd(nc, in_maps, core_ids=list(range(B)),
                                          trace=trace, **kw)
    out = np.stack([r["out"].reshape(O, H, W) for r in res.results])
    return out.astype(np.float32), res


def kernel(**inputs):
    return run(inputs)[0]


# revision 4
# speedup vs baseline: 1.2029x; 1.0090x over previous
"""Modulated deformable conv v2 (B=8, C=O=256, H=W=64, 3x3) on 8 trn2 NeuronCores.

v2.1: bf16 datapath + host-precomputed gather indices / corner weights.

Strategy: data-parallel over batch (1 image per core). Per core:
  - host marshals the image into a "patch array" in3[j] = concat of padded
    [HW, C] rows (j, j+1, j+64, j+65) in bf16, so one contiguous 2KB DMA
    descriptor fetches the full 2x2 bilinear patch for all 256 channels.
  - host also computes the int16 gather row indices (jw, 16-partition
    wrapped, iteration-major) and the 4 folded corner weights (bilinear
    frac * validity * modulation mask, W-layout bf16).
  - device: gpsimd dma_gather streams patch rows (HBM->SBUF) 4 iterations
    (2048 rows) per call, DVE applies the 4 per-row corner weights
    (1 tensor_scalar + 3 scalar_tensor_tensor per 128-position group, all
    bf16, no drains), PE transposes [p,c]->[c,p] via identity matmuls
    (bf16 PSUM), ACT copies PSUM->SBUF into per-chunk [128, 9*1024] tiles,
    PE runs one N=512 einsum per 512-position chunk with PSUM
    accumulation over (k, cb), ACT folds the bias, HWDGE DMAs f32 output.
"""

import numpy as np
from contextlib import ExitStack

import concourse.bacc as bacc
import concourse.bass as bass
import concourse.mybir as mybir
from concourse import bass_utils
from concourse.library_config import mlp

AP = bass.AP
F32 = mybir.dt.float32
BF16 = mybir.dt.bfloat16
I16 = mybir.dt.int16

# problem constants (hardcoded per contract)
B = 8
C = 256
O = 256
H = W = 64
HW = 4096
K2 = 9

# tiling
NCH = 8          # spatial chunks
CHP = 512        # positions per chunk
NG = 4           # 128-position groups per chunk
NITER = NCH * K2 # 72 macro iterations (chunk-major, then k)
NGI = NITER * NG # 288 (i,g) steps
GB = 2           # iterations per dma_gather batch
NBATCH = NITER // GB  # 18

PADLO = 65       # leading pad rows in the padded [HW, C] image
R2 = 4292        # padded image rows
R3 = 4232        # patch-array rows (4225 used)


def _np_bf16():
    import ml_dtypes
    return ml_dtypes.bfloat16


# ---------------------------------------------------------------------------
# bass program
# ---------------------------------------------------------------------------

def build_nc():
    nc = bacc.Bacc("TRN2", detect_race_conditions=False)

    in3 = nc.dram_tensor("in3", [R3, 1024], BF16, kind="ExternalInput")
    wT = nc.dram_tensor("wT", [2304, 256], BF16, kind="ExternalInput")
    identm = nc.dram_tensor("identm", [128, 128], BF16, kind="ExternalInput")
    biasm = nc.dram_tensor("biasm", [128, 2], F32, kind="ExternalInput")
    jwH = nc.dram_tensor("jwH", [128, 2304], I16, kind="ExternalInput")
    wcH = nc.dram_tensor("wcH", [128, 4 * 288], F32, kind="ExternalInput")
    outT = nc.dram_tensor("out", [256, 4096], F32, kind="ExternalOutput")

    with ExitStack() as ctx:
        ec = ctx.enter_context

        # sbuf
        gdst = [ec(nc.sbuf_tensor(f"gdst{j}", [128, GB * 4096], BF16))
                for j in range(4)]
        s_rows = [ec(nc.sbuf_tensor(f"srows{j}", [128, 1024], BF16)) for j in range(2)]
        st_sb = [ec(nc.sbuf_tensor(f"stsb{j}", [128, 256], BF16)) for j in range(2)]
        out_sb = [ec(nc.sbuf_tensor(f"outsb{j}", [128, 1024], F32)) for j in range(2)]
        w_sb = ec(nc.sbuf_tensor("wsb", [128, 4608], BF16))
        ident_sb = ec(nc.sbuf_tensor("identsb", [128, 128], BF16))
        bias_sb = ec(nc.sbuf_tensor("biassb", [128, 2], F32))
        tmpA = [[ec(nc.sbuf_tensor(f"tmpA{j}_{g}", [128, 256], BF16))
                 for g in range(NG)] for j in range(3)]
        tmpB = [ec(nc.sbuf_tensor(f"tmpB{g}", [128, 256], BF16)) for g in range(NG)]
        wcor = [ec(nc.sbuf_tensor(f"wc{q}", [128, 288], F32)) for q in range(4)]
        jw = ec(nc.sbuf_tensor("jw", [128, 2304], I16))

        # psum: transposes in bf16 (must match lhsT dtype), einsum accum f32
        psT = [ec(nc.psum_tensor(f"psT{j}", [128, 1024], BF16)) for j in range(2)]
        psE = [ec(nc.psum_tensor(f"psE{g}", [128, 512], F32)) for g in range(NG)]

        sem_ld = ec(nc.semaphore("sem_ld"))
        sem_ac0 = ec(nc.semaphore("sem_ac0"))
        sem_prep2 = ec(nc.semaphore("sem_prep2"))
        sem_gat = ec(nc.semaphore("sem_gat"))
        sem_dve = ec(nc.semaphore("sem_dve"))
        sem_pet = ec(nc.semaphore("sem_pet"))
        sem_act = ec(nc.semaphore("sem_act"))
        sem_pee = ec(nc.semaphore("sem_pee"))
        sem_epi = ec(nc.semaphore("sem_epi"))
        sem_out = ec(nc.semaphore("sem_out"))

        # ---- AP helpers (flat element offsets) ----
        def sb(t, off, free, count=128, pstep=None):
            if pstep is None:
                pstep = t.shape[1] if len(t.shape) == 2 else int(np.prod(t.shape[1:]))
            return AP(t, off, [[pstep, count], [1, free]])

        def scl(t, col):
            return AP(t, col, [[t.shape[1], 128], [1, 1]])

        loads = [
            (sb(ident_sb, 0, 128), AP(identm, 0, [[128, 128], [1, 128]])),
            (sb(bias_sb, 0, 2), AP(biasm, 0, [[2, 128], [1, 2]])),
            (sb(jw, 0, 2304), AP(jwH, 0, [[2304, 128], [1, 2304]])),
        ]
        for q in range(4):
            loads.append((sb(wcor[q], 0, 288),
                          AP(wcH, q * 288, [[4 * 288, 128], [1, 288]])))
        for kcb in range(18):
            loads.append((sb(w_sb, kcb * 256, 256),
                          AP(wT, kcb * 128 * 256, [[256, 128], [1, 256]])))
        n_loads = len(loads)

        glast = {}  # chunk -> gi of last transpose-copy step
        for c in range(NCH):
            glast[(c * K2 + (K2 - 1)) * NG + (NG - 1)] = c

        with nc.Block() as block:

            @block.sync
            def _(sync):
                for dst, src in loads:
                    sync.dma_start(dst, src).then_inc(sem_ld, 16)
                for c in range(NCH):
                    sync.wait_ge(sem_epi, 8 * (c + 1))
                    for ob in range(2):
                        dst = AP(outT, ob * 128 * 4096 + c * 512,
                                 [[4096, 128], [1, 512]])
                        src = sb(out_sb[c % 2], ob * 512, 512)
                        sync.dma_start(dst, src).then_inc(sem_out, 16)

            @block.gpsimd
            def _(gp):
                gp.load_library(mlp)
                gp.wait_ge(sem_ld, 16 * n_loads)
                in3_ap = AP(in3, 0, [[1024, R3], [1, 1024]])
                for b in range(NBATCH):
                    if b >= 4:
                        # gdst[b%4] reuse: DVE done with iters of batch b-4
                        gp.wait_ge(sem_dve, 4 * GB * (b - 3))
                    dst = AP(gdst[b % 4], 0,
                             [[GB * 4096, 128], [1024, 4 * GB], [1, 1024]])
                    idx = AP(jw, b * GB * 32, [[2304, 128], [1, GB * 32]])
                    gp.dma_gather(dst, in3_ap, idx, GB * CHP, GB * CHP, 1024,
                                  prepare_only=True, sem=sem_gat).then_inc(
                        sem_prep2, 1)
                    gp.wait_ge(sem_prep2, b + 1)
                    gp.trigger_dma(count=1)

            @block.vector
            def _(v):
                A = mybir.AluOpType
                v.wait_ge(sem_ld, 16 * n_loads)

                # ---- per-(i,g) corner-weight application ----
                # 4 independent groups per stage; RAW pairs are 3 ops apart,
                # so no explicit drains needed (DVE pipe is 8 slices deep).
                for i in range(NITER):
                    c, k = divmod(i, K2)
                    v.wait_ge(sem_gat, 16 * (i // GB + 1))
                    if i >= 2:
                        v.wait_ge(sem_pet, NG * (i - 1))
                    cols = [k * 32 + c * NG + g for g in range(NG)]
                    base = (i % GB) * 4096
                    gsl = [[AP(gdst[(i // GB) % 4], base + g * 1024 + q * 256,
                               [[GB * 4096, 128], [1, 256]]) for q in range(4)]
                           for g in range(NG)]
                    a_ = [sb(t, 0, 256) for t in tmpA[i % 3]]
                    b_ = [sb(t, 0, 256) for t in tmpB]
                    dstS = [sb(s_rows[i % 2], g * 256, 256) for g in range(NG)]
                    v.wait_ge(sem_ac0, i + 1)
                    for g in range(NG):
                        v.scalar_tensor_tensor(b_[g], gsl[g][1],
                                               scl(wcor[1], cols[g]), a_[g],
                                               A.mult, A.add)
                    for g in range(NG):
                        v.scalar_tensor_tensor(a_[g], gsl[g][2],
                                               scl(wcor[2], cols[g]), b_[g],
                                               A.mult, A.add)
                    for g in range(NG):
                        v.scalar_tensor_tensor(dstS[g], gsl[g][3],
                                               scl(wcor[3], cols[g]), a_[g],
                                               A.mult, A.add).then_inc(sem_dve)

            @block.tensor
            def _(te):
                te.wait_ge(sem_ld, 16 * n_loads)

                def emit_einsum(gg):
                    i2, g2 = divmod(gg, NG)
                    c2, k2 = divmod(i2, K2)
                    te.wait_ge(sem_act, gg + 1)
                    if k2 == 0 and c2 >= 1:
                        # psE[g2] bank reused across chunks; wait for the
                        # previous chunk's epilogue to finish reading it
                        te.wait_ge(sem_epi, 8 * c2)
                    last = None
                    for ob in range(2):
                        for cb in range(2):
                            lhs = AP(w_sb, (k2 * 2 + cb) * 256 + ob * 128,
                                     [[4608, 128], [1, 128]])
                            rhs = AP(st_sb[gg % 2], cb * 128, [[256, 128], [1, 128]])
                            dst = AP(psE[g2], ob * 128, [[512, 128], [1, 128]])
                            last = te.matmul(dst, lhs, rhs,
                                             start=(k2 == 0 and ob == 0
                                                    and cb == 0),
                                             stop=(k2 == K2 - 1 and ob == 1
                                                   and cb == 1))
                    last.then_inc(sem_pee)

                for i in range(NITER):
                    for g in range(NG):
                        gi = NG * i + g
                        te.wait_ge(sem_dve, gi + 1)
                        if gi >= 2:
                            te.wait_ge(sem_act, gi - 1)
                        pg = gi % 2
                        last = None
                        for cb in range(2):
                            src = AP(s_rows[i % 2], g * 256 + cb * 128,
                                     [[1024, 128], [1, 128]])
                            dst = AP(psT[pg], cb * 128, [[1024, 128], [1, 128]])
                            last = te.transpose(dst, src, sb(ident_sb, 0, 128))
                        last.then_inc(sem_pet)
                        if gi >= 1:
                            emit_einsum(gi - 1)
                emit_einsum(NGI - 1)

            @block.scalar
            def _(sc):
                IDENT = mybir.ActivationFunctionType.Identity

                def corner0(j):
                    # a_g = gathered corner-0 block * w00[p], into tmpA[j%2]
                    cj, kj = divmod(j, K2)
                    sc.wait_ge(sem_gat, 16 * (j // GB + 1))
                    if j >= 3:
                        # tmpA[j%3] reuse: DVE stage1 of iter j-3 done
                        sc.wait_ge(sem_dve, NG * (j - 2))
                    a0 = None
                    for g0 in range(NG):
                        col0 = kj * 32 + cj * NG + g0
                        src0 = AP(gdst[(j // GB) % 4],
                                  (j % GB) * 4096 + g0 * 1024,
                                  [[GB * 4096, 128], [1, 256]])
                        a0 = sc.activation(sb(tmpA[j % 3][g0], 0, 256),
                                           src0, IDENT,
                                           scale=scl(wcor[0], col0))
                    a0.then_inc(sem_ac0)

                corner0(0)
                corner0(1)
                for gi in range(NGI):
                    i, g = divmod(gi, NG)
                    sc.wait_ge(sem_pet, gi + 1)
                    if gi >= 2:
                        sc.wait_ge(sem_pee, gi - 1)
                    sc.activation(sb(st_sb[gi % 2], 0, 256),
                                  AP(psT[gi % 2], 0, [[1024, 128], [1, 256]]),
                                  IDENT).then_inc(sem_act)
                    if g == NG - 1 and i + 2 < NITER:
                        # corner-0 two iters ahead, BEFORE any epilogue wait
                        corner0(i + 2)
                    c = glast.get(gi)
                    if c is not None:
                        if c >= 2:
                            sc.wait_ge(sem_out, 32 * (c - 1))
                        for g2 in range(NG):
                            sc.wait_ge(sem_pee, (c * K2 + K2 - 1) * NG + g2 + 1)
                            for ob in range(2):
                                sc.activation(
                                    sb(out_sb[c % 2], ob * 512 + g2 * 128, 128),
                                    AP(psE[g2], ob * 128,
                                       [[512, 128], [1, 128]]),
                                    IDENT, bias=scl(bias_sb, ob),
                                ).then_inc(sem_epi)

    nc.compile()
    return nc


# ---------------------------------------------------------------------------
# host marshalling
# ---------------------------------------------------------------------------

def _to_W(f):
    # f [9, 4096] -> [128, 288]; fW[p%128, k*32 + p//128] = f[k, p]
    return np.ascontiguousarray(
        f.reshape(9, 32, 128).transpose(2, 0, 1).reshape(128, 288))


def _to_I72(f):
    # f [72, 512] (iter-major, i = c*9+k) -> wrapped [128, 2304];
    # fI[r, i*32+t] = f[i, t*16 + r%16]
    a = f.reshape(72, 32, 16).transpose(2, 0, 1).reshape(16, 2304)
    return np.ascontiguousarray(np.tile(a, (8, 1)))


def marshal(inputs):
    bf16 = _np_bf16()
    inp = np.asarray(inputs["input"], np.float32)
    off = np.asarray(inputs["offset"], np.float32)
    msk = np.asarray(inputs["mask"], np.float32)
    wgt = np.asarray(inputs["weight"], np.float32)
    bias = np.asarray(inputs["bias"], np.float32)

    wT = np.ascontiguousarray(
        wgt.reshape(O, C, K2).transpose(2, 1, 0).reshape(2304, 256)).astype(bf16)
    identm = np.eye(128, dtype=np.float32).astype(bf16)
    biasm = np.ascontiguousarray(bias.reshape(2, 128).T)

    # sample coordinates: y = off_y + base_y, base_y = ho - 1 + k//3
    ho = np.arange(HW, dtype=np.float32) // 64
    wo = np.arange(HW, dtype=np.float32) % 64
    ks = np.arange(K2, dtype=np.float32)
    by = ho[None, :] - 1.0 + (ks // 3)[:, None]     # [9, 4096]
    bx = wo[None, :] - 1.0 + (ks % 3)[:, None]

    in_maps = []
    for b in range(B):
        img = inp[b].transpose(1, 2, 0).reshape(HW, C)
        in2p = np.zeros((R2, C), np.float32)
        in2p[PADLO:PADLO + HW] = img
        in3 = np.zeros((R3, 1024), np.float32)
        n = HW + 2 * PADLO - 1  # 4225 usable rows
        in3[:n, 0:256] = in2p[0:n]
        in3[:n, 256:512] = in2p[1:n + 1]
        in3[:n, 512:768] = in2p[64:n + 64]
        in3[:n, 768:1024] = in2p[65:n + 65]

        off_y = off[b, 0::2].reshape(K2, HW)
        off_x = off[b, 1::2].reshape(K2, HW)
        m = msk[b].reshape(K2, HW)

        y = off_y + by
        x = off_x + bx
        y0 = np.floor(y)
        x0 = np.floor(x)
        ly = (y - y0).astype(np.float32)
        lx = (x - x0).astype(np.float32)
        hy = 1.0 - ly
        hx = 1.0 - lx
        vy0 = ((y0 >= 0) & (y0 <= 63)).astype(np.float32)
        vy1 = ((y0 >= -1) & (y0 <= 62)).astype(np.float32)
        vx0 = ((x0 >= 0) & (x0 <= 63)).astype(np.float32)
        vx1 = ((x0 >= -1) & (x0 <= 62)).astype(np.float32)
        w00 = hy * hx * vy0 * vx0 * m
        w01 = hy * lx * vy0 * vx1 * m
        w10 = ly * hx * vy1 * vx0 * m
        w11 = ly * lx * vy1 * vx1 * m

        py = np.clip(y0, -1, 63)
        px = np.clip(x0, -1, 63)
        J = (64.0 * py + px + float(PADLO)).astype(np.float32)
        # iteration-major [72, 512] with i = c*9 + k
        J72 = J.reshape(9, 8, 512).transpose(1, 0, 2).reshape(72, 512)

        wcH = np.concatenate(
            [_to_W(w) for w in (w00, w01, w10, w11)], axis=1)
        im = {
            "in3": in3.astype(bf16),
            "jwH": _to_I72(J72).astype(np.int16),
            "wcH": np.ascontiguousarray(wcH, np.float32),
            "wT": wT, "identm": identm, "biasm": biasm,
        }
        in_maps.append(im)
    return in_maps


_NC_CACHE = {}


def _get_nc():
    if "nc" not in _NC_CACHE:
        _NC_CACHE["nc"] = build_nc()
    return _NC_CACHE["nc"]


def run(inputs, trace=False, **kw):
    nc = _get_nc()
    in_maps = marshal(inputs)
    res = bass_utils.run_bass_kernel_spmd(nc, in_maps, core_ids=list(range(B)),
                                          trace=trace, **kw)
    out = np.stack([r["out"].reshape(O, H, W) for r in res.results])
    return out.astype(np.float32), res


def kernel(**inputs):
    return run(inputs)[0]
